# revision 18
# baseline (speedup 1.0000x reference)
"""Trainium2 Bass kernel for the all-pairs spring-energy sum (EnergyLossVectorized).

Contract: kernel(**inputs) takes FULL unsharded inputs (p [32768,2] f32,
edge_attr [E,2] f32, src/dst [E] i32 with E = 64*512*511), returns the FULL
scalar output, distributing across 8 NeuronCores internally.

Strategy: src/dst produced by the reference's setup_inputs() are the
deterministic all-directed-pairs (i != j) indices per graph, in i-major
order.  We verify that structure on the host (falling back to a straight
numpy evaluation if it ever doesn't hold) and then compute the energy with
a gather-free formulation:

  For each graph g (512 nodes), the 512x512 grid D2[i,j] = |p_i - p_j|^2 is
  computed on the tensor engine as a K=8 matmul:
     D2 = sum_f PL[f,i] * PR[f,j]
     PL features: [ x,  y,  rhi, rmid, rlo, 1, 1, 1 ]
     PR features: [-2x, -2y, 1,   1,   1,   rhi, rmid, rlo ]
  where r = x^2 + y^2 is carried as three bf16 limbs (rhi+rmid+rlo) so the
  PSUM result equals |p_i - p_j|^2 to ~fp32 accuracy (no catastrophic
  cancellation), guaranteeing D2 >= -1e-5 so sqrt(D2 + EPS) is NaN-free.

  edge_attr (l, k) is re-laid-out on the host into per-graph [512,512] bf16
  grids with k=0 on the diagonal (diagonal contributes k/2*(sqrt(eps)-l)^2*0
  = 0).  Per tile [128 i x 512 j]:
     s  = sqrt(D2 + EPS)        (scalar engine, PSUM -> SBUF bf16)
     u  = s - l                 (DVE)
     u2 = u * u                 (DVE / GPSIMD, alternating)
     e  = u2 * k                (DVE)
     acc[1,512] += ones[128]^T @ e   (PE matmul, PSUM accumulate)
  Finally acc is reduced to a scalar and scaled by 0.5 on device; the host
  sums the 8 per-core partials.

Memory traffic per core: 8 graphs * 2 grids * 512KB = 8.4 MB (bf16), which
is the memory-bound roofline target (~24 us at ~350 GB/s).
"""

import os
import sys

import numpy as np

for _p in ("/opt/trn_rl_repo", "/root/.axon_site/_ro/trn_rl_repo"):
    if os.path.isdir(_p) and _p not in sys.path:
        sys.path.insert(0, _p)

import ml_dtypes

bf16 = ml_dtypes.bfloat16

NUM_GRAPHS = 64
N = 512                      # nodes per graph
NCORES = 8
GPC = NUM_GRAPHS // NCORES   # graphs per core = 8
PB = 128                     # partition block (i-tile)
EPS = 1e-5                   # sqrt clamp; D2 >= -1e-5 guaranteed by 3-limb r

# u2 = u*u engine assignment per (g,t) tile index: split between DVE and
# GPSIMD to balance engine load (DVE also does sub and mul-k).
SQ_ON_GPSIMD_EVERY = 3  # every 3rd tile's square goes to gpsimd


def _build_nc(gpc=GPC, n=N, pb=PB, debug=False):
    """Build + compile the per-core Bass program (SPMD, same on all cores)."""
    import concourse.bass as bass
    import concourse.tile as tile
    from concourse import bacc, mybir

    tb = n // pb             # i-tiles per graph
    fdt = mybir.dt.float32
    bdt = mybir.dt.bfloat16
    AF = mybir.ActivationFunctionType

    nc = bacc.Bacc("TRN2", target_bir_lowering=False, debug=debug,
                   num_devices=NCORES)

    lg_d = nc.dram_tensor("lg", [gpc, tb, pb, n], bdt, kind="ExternalInput")
    kg_d = nc.dram_tensor("kg", [gpc, tb, pb, n], bdt, kind="ExternalInput")
    pl_d = nc.dram_tensor("plin", [64, 4 * n], bdt, kind="ExternalInput")
    pr_d = nc.dram_tensor("prin", [64, 4 * n], bdt, kind="ExternalInput")
    out_d = nc.dram_tensor("out", [1, 1], fdt, kind="ExternalOutput")

    lg = lg_d.ap()
    kg = kg_d.ap()

    # plt/prt [64, 4n] (host-precomputed): quadrant g' in {0,1}, free quarter
    # gg in {0..3}, graph g = 4*g' + gg; rows 0-7 of the quadrant hold the
    # K=8 matmul features of that graph:
    #   plt: [x, y, rhi, rmid, rlo, 1, 1, 1]
    #   prt: [-2x, -2y, 1, 1, 1, rhi, rmid, rlo]
    # where r = x^2+y^2 of the bf16-rounded coords, carried as 3 bf16 limbs
    # so D2 = lhsT.T@rhs >= -1e-5 exactly (no cancellation blowup).
    # (matmul operands must start at partition base 0/32/64.)

    with tile.TileContext(nc) as tc:
        with (
            tc.tile_pool(name="const", bufs=1) as const,
            tc.tile_pool(name="lk", bufs=3) as lk,
            tc.tile_pool(name="work", bufs=4) as work,
            tc.tile_pool(name="psum", bufs=4, space="PSUM") as psum,
            tc.tile_pool(name="accp", bufs=1, space="PSUM") as accp,
        ):
            # ---- load host-precomputed matmul operands ----
            plt = const.tile([64, 4 * n], bdt)
            prt = const.tile([64, 4 * n], bdt)
            nc.sync.dma_start(plt[:], pl_d.ap())
            nc.sync.dma_start(prt[:], pr_d.ap())

            ones_col = const.tile([pb, 1], bdt)
            nc.vector.memset(ones_col[:], 1.0)
            eps_col = const.tile([pb, 1], fdt)
            nc.vector.memset(eps_col[:], EPS)

            acc = accp.tile([1, n], fdt)

            # ---- main loop over graphs ----
            n_tiles = gpc * tb
            idx = 0
            for g in range(gpc):
                g_, gg = divmod(g, 4)
                lt = lk.tile([pb, tb, n], bdt, tag="L")
                nc.sync.dma_start(lt[:], lg[g].rearrange("t p j -> p t j"))
                kt = lk.tile([pb, tb, n], bdt, tag="K")
                nc.sync.dma_start(kt[:], kg[g].rearrange("t p j -> p t j"))
                for t in range(tb):
                    ps = psum.tile([pb, n], fdt)
                    nc.tensor.matmul(
                        ps[:],
                        plt[32 * g_:32 * g_ + 8,
                            gg * n + t * pb: gg * n + (t + 1) * pb],
                        prt[32 * g_:32 * g_ + 8, gg * n:(gg + 1) * n],
                        start=True, stop=True,
                    )
                    s = work.tile([pb, n], bdt, tag="s")
                    nc.scalar.activation(s[:], ps[:], AF.Sqrt, bias=eps_col[:])
                    u = work.tile([pb, n], bdt, tag="u")
                    nc.vector.tensor_sub(u[:], s[:], lt[:, t, :])
                    u2 = work.tile([pb, n], bdt, tag="u2")
                    if idx % SQ_ON_GPSIMD_EVERY == SQ_ON_GPSIMD_EVERY - 1:
                        nc.gpsimd.tensor_mul(u2[:], u[:], u[:])
                    else:
                        nc.vector.tensor_mul(u2[:], u[:], u[:])
                    e = work.tile([pb, n], bdt, tag="e")
                    nc.vector.tensor_mul(e[:], u2[:], kt[:, t, :])
                    nc.tensor.matmul(
                        acc[:], ones_col[:], e[:],
                        start=(idx == 0), stop=(idx == n_tiles - 1),
                        skip_group_check=True,
                    )
                    idx += 1

            # ---- final reduction to a scalar ----
            acc_sb = const.tile([1, n], fdt)
            nc.vector.tensor_copy(acc_sb[:], acc[:])
            tot = const.tile([1, 1], fdt)
            nc.vector.tensor_reduce(
                tot[:], acc_sb[:], axis=mybir.AxisListType.X,
                op=mybir.AluOpType.add,
            )
            half = const.tile([1, 1], fdt)
            nc.vector.tensor_scalar_mul(half[:], tot[:], 0.5)
            nc.sync.dma_start(out_d.ap(), half[:])

    nc.compile()
    return nc


_NC_CACHE = {}


def _get_nc(gpc=GPC, n=N, pb=PB):
    key = (gpc, n, pb)
    if key not in _NC_CACHE:
        _NC_CACHE[key] = _build_nc(gpc, n, pb)
    return _NC_CACHE[key]


def _expected_pairs(num_graphs, n):
    i = np.repeat(np.arange(n, dtype=np.int64), n)
    j = np.tile(np.arange(n, dtype=np.int64), n)
    keep = i != j
    si, sj = i[keep], j[keep]
    off = (np.arange(num_graphs, dtype=np.int64) * n)[:, None]
    src = (off + si[None, :]).reshape(-1)
    dst = (off + sj[None, :]).reshape(-1)
    return src.astype(np.int32), dst.astype(np.int32)


def _structure_ok(src, dst):
    if src.shape != (NUM_GRAPHS * N * (N - 1),):
        return False
    esrc, edst = _expected_pairs(NUM_GRAPHS, N)
    return np.array_equal(src, esrc) and np.array_equal(dst, edst)


def _fallback_numpy(p, edge_attr, src, dst):
    start = p[src].astype(np.float64)
    end = p[dst].astype(np.float64)
    t12 = ((start - end) ** 2).sum(axis=1)
    l = edge_attr[:, 0].astype(np.float64)
    k = edge_attr[:, 1].astype(np.float64)
    energy = k / 2.0 * (t12 + l * l - 2.0 * l * np.sqrt(t12))
    return np.float32(energy.sum())


def _build_plt_prt(p_core, gpc=GPC, n=N):
    """p_core [gpc*n, 2] f32 -> (plt, prt) [64, 4n] bf16 matmul operands."""
    xb = p_core.reshape(gpc, n, 2).astype(bf16)          # bf16-rounded coords
    xf = xb[..., 0].astype(np.float32)
    yf = xb[..., 1].astype(np.float32)
    r = xf * xf + yf * yf
    rhi = r.astype(bf16)
    r1 = r - rhi.astype(np.float32)
    rmid = r1.astype(bf16)
    r2 = r1 - rmid.astype(np.float32)
    rlo = r2.astype(bf16)
    plt = np.ones((64, 4 * n), dtype=bf16)
    prt = np.ones((64, 4 * n), dtype=bf16)
    feats_l = [xb[..., 0], xb[..., 1], rhi, rmid, rlo]
    feats_r = [(xb[..., 0] * bf16(-2.0)), (xb[..., 1] * bf16(-2.0)),
               None, None, None, rhi, rmid, rlo]
    for g in range(gpc):
        g_, gg = divmod(g, 4)
        cols = slice(gg * n, (gg + 1) * n)
        for f, arr in enumerate(feats_l):
            plt[32 * g_ + f, cols] = arr[g]
        for f, arr in enumerate(feats_r):
            if arr is not None:
                prt[32 * g_ + f, cols] = arr[g]
    return plt, prt


def _build_grids(edge_attr):
    """edge_attr [E,2] f32 -> L, K bf16 arrays [NCORES, GPC, TB, PB, N]."""
    tb = N // PB
    ea = edge_attr.astype(bf16).reshape(NUM_GRAPHS, N * (N - 1), 2)
    offdiag = (~np.eye(N, dtype=bool)).reshape(-1)
    grid = np.zeros((2, NUM_GRAPHS, N * N), dtype=bf16)
    grid[0][:, offdiag] = ea[:, :, 0]
    grid[1][:, offdiag] = ea[:, :, 1]
    L = grid[0].reshape(NCORES, GPC, tb, PB, N)
    K = grid[1].reshape(NCORES, GPC, tb, PB, N)
    return L, K


def kernel(p, edge_attr, src, dst):
    p = np.ascontiguousarray(np.asarray(p, dtype=np.float32))
    edge_attr = np.ascontiguousarray(np.asarray(edge_attr, dtype=np.float32))
    src = np.asarray(src, dtype=np.int32)
    dst = np.asarray(dst, dtype=np.int32)

    if not _structure_ok(src, dst):
        return _fallback_numpy(p, edge_attr, src, dst)

    from concourse.bass_utils import run_bass_kernel_spmd

    L, K = _build_grids(edge_attr)
    pcs = p.reshape(NCORES, GPC * N, 2)

    nc = _get_nc()
    in_maps = []
    for c in range(NCORES):
        plt, prt = _build_plt_prt(pcs[c])
        in_maps.append({"lg": L[c], "kg": K[c], "plin": plt, "prin": prt})
    res = run_bass_kernel_spmd(nc, in_maps, list(range(NCORES)))
    total = sum(float(res.results[c]["out"][0, 0]) for c in range(NCORES))
    return np.float32(total)


if __name__ == "__main__":
    # smoke: build the program only
    nc = _get_nc()
    print("compiled ok")
